# revision 15
# baseline (speedup 1.0000x reference)
"""Trainium2 Bass kernel for CNF log-prob (nn_CNF_86019605004441).

Reference computation (per batch row b of B=32768):
  Integrate (z, logp) from t=1 to t=0 with 4 fixed RK4 steps (steps=5 ->
  4 intervals). Each RK4 stage evaluates
     f(t, z)   = tanh([z, ctx, t] @ W1 + b1) @ W2 + b2
     div(t, z) = eps^T J eps  (Hutchinson, exact via jvp)
  With h = tanh(a):  div = sum_j (1 - h_j^2) * t1_j * v_j
     where t1 = eps @ W1[:16]  and  v = eps @ W2^T  are eval-independent.
  Using u = t1*v and U = sum_j u_j:  div = U - S,  S = sum_j h_j^2 u_j.
  logp(x) = -0.5*sum(z1^2) - 0.5*16*log(2pi) + delta_logp.

Sharding: pure data parallel, batch 32768 -> 8 cores x 4096 rows.

On-core layout (features on partitions, batch on the free axis, fp16 state):
  inT [81, 4096] f16: rows 0-15 z (current eval input), 16-79 ctx, 80 ones.
  mm1 stationary per (eval i, chunk c): W1v[:, i*4+c, :] [81,128] f16; row 80
      carries beta = t_i*W1_time + b1 + delta_i*(W1z.T@b2) (time feature, b1,
      deferred-b2 folded in -> ACT does a pure tanh).
  zS/zSacc [17, 4096] f16: rows 0-15 z state, row 16 logp (init
      U - 0.5*16*log(2pi); the -dt*U divergence constant telescopes to +U).

Main loop: one flat software pipeline over 128 unit-evals E_k =
  (step s, half hb, stage, unit uu) of 512 batch cols each. Per E_k:
    PE : 4x mm1 into pa psum (2 tiles of [128,2,512]);  4x f-mm2 with a
         STACKED stationary [w*W2 | 0 0 | alpha*W2] ([128,34] f16) + 4x
         div-mm2 ([w, alpha] cols) -> one CUR psum region of 34 rows
         (0-15 w*f, 16 w*S, 17 a*S, 18-33 a*f), 2 units/bank (base 0/64).
    ACT: 2x tanh [128,2,512] psum->sbuf f16; 1x Copy CUR [34,512] -> curS
         (some rotated to DVE for balance).
    DVE: hh = h*h, q = hh*u in place (fp16 2x-mode); inT z-rows =
         curS[18:34] + zS (the one partition-offset read, DVE-proven).
    Pool(GPSIMD, sbuf-only, partition-aligned only):
         zSacc[0:17] = curS[0:17] + (zS | zSacc)     [w-scaled RK4 accum]
         stage3: zS = curS[0:17] + zSacc, inT = curS[0:16] + zSacc.
  Emission interleaves E_k's mm1/tanh/hh/q with E_{k-1}'s f/div/copy/adds so
  every engine queue stays one unit ahead of its dependencies.
Finalize: zsq = (z1 - b2)^2 ; colsum via ones-matmul ; out = -0.5*colsum
  + zS[16].
"""

import sys
import numpy as np

for _p in ("/opt/trn_rl_repo",):
    if _p not in sys.path:
        sys.path.insert(0, _p)

DIM, COND, HID = 16, 64, 512
B, NCORES = 32768, 8
NB = B // NCORES          # 4096 batch rows per core
P = 128                   # partitions
NCH = HID // P            # 4 hidden chunks
NJ = NB // 512            # 8 batch column groups
KIN = DIM + COND + 1      # 81 stationary rows (z, ctx, ones)
ONE_R = KIN - 1           # 80 = ones/beta row
SD_P = 2 * DIM + 1        # 33 = state rows (z 0-15, zero pad, logp 32)
DV = 2 * DIM              # 32 = logp row
CW = 2 * DIM + 1          # 33 = CUR region rows (w*f 0-15, pad, w*S 32)
NSTEPS, NSTAGE = 4, 4
NEV = NSTEPS * NSTAGE     # 16 rhs evaluations
LOG2PI = float(np.log(2.0 * np.pi))


def _schedule():
    """Per-eval (t, alpha_next, w, delta) for classic RK4, t:1->0, dt=-0.25."""
    ts = np.linspace(1.0, 0.0, NSTEPS + 1)
    evs = []
    for s in range(NSTEPS):
        t0 = float(ts[s])
        dt = float(ts[s + 1] - ts[s])
        dbase = s * dt
        evs.append(dict(t=t0, alpha=dt / 2, w=dt / 6, delta=dbase))
        evs.append(dict(t=t0 + dt / 2, alpha=dt / 2, w=dt / 3, delta=dbase + dt / 2))
        evs.append(dict(t=t0 + dt / 2, alpha=dt, w=dt / 3, delta=dbase + dt / 2))
        evs.append(dict(t=t0 + dt, alpha=None, w=dt / 6, delta=dbase + dt))
    return evs


def prep_host_inputs(x, context, eps, W1, b1, W2, b2):
    """Host-side layout prep (transposes + per-eval stationary weight packing).

    Returns the in_map dict for one core given that core's batch slice."""
    evs = _schedule()
    W1 = np.asarray(W1, np.float32)
    b1 = np.asarray(b1, np.float32)
    W2 = np.asarray(W2, np.float32)
    b2 = np.asarray(b2, np.float32)

    gz = W1[:DIM].T @ b2  # [512], the z-column correction for deferred b2
    W1v = np.zeros((KIN, NEV * NCH, P), np.float32)
    for i, ev in enumerate(evs):
        for c in range(NCH):
            sl = slice(c * P, (c + 1) * P)
            v = i * NCH + c
            W1v[0:DIM, v, :] = W1[0:DIM, sl]
            W1v[DIM:ONE_R, v, :] = W1[DIM : DIM + COND, sl]
            W1v[ONE_R, v, :] = (
                ev["t"] * W1[DIM + COND, sl] + b1[sl] + ev["delta"] * gz[sl]
            )

    # f-mm2 stationary per eval, w_i prescaled (cols 0-15 w_i*W2, cols
    # 16-32 zero); div-mm2 one col [w_i] -> psum row 32 (32-aligned out).
    W2fa = np.zeros((P, NEV, NCH, CW), np.float16)
    divW = np.zeros((P, NEV, 1), np.float16)
    w2c = W2.reshape(NCH, P, DIM).transpose(1, 0, 2)  # [P, NCH, DIM]
    for i, ev in enumerate(evs):
        W2fa[:, i, :, 0:DIM] = (w2c * ev["w"]).astype(np.float16)
        divW[:, i, 0] = np.float16(ev["w"])

    W2T = np.ascontiguousarray(W2.T)  # [16, 512] for the v = eps@W2^T matmul
    b2c = (NSTEPS * (-0.25)) * b2.reshape(DIM, 1).astype(np.float32)

    def core_map(xs, cs, es):
        initT = np.zeros((KIN, NB), np.float32)
        initT[0:DIM] = xs.T
        initT[DIM:ONE_R] = cs.T
        initT[ONE_R] = 1.0
        return {
            "initT": initT.astype(np.float16),             # [81, NB]
            "epsT": np.ascontiguousarray(es.T).astype(np.float16),
            "onesZ": np.ones((DIM, 1), np.float16),
            "W1v": W1v.astype(np.float16),                 # [81, 64, 128]
            "W2T": W2T.astype(np.float16),                 # [16, 512]
            "W1z": W1[0:DIM].astype(np.float16),           # [16, 512]
            "W2fa": W2fa,                                  # [128, 16, 4, 34]
            "divW": divW,                                  # [128, 16, 1]
            "b2c": b2c,                                    # [16, 1]
        }

    return [
        core_map(
            np.asarray(x, np.float32)[i * NB : (i + 1) * NB],
            np.asarray(context, np.float32)[i * NB : (i + 1) * NB],
            np.asarray(eps, np.float32)[i * NB : (i + 1) * NB],
        )
        for i in range(NCORES)
    ]


def build(nc, tc, ctx):
    """Emit the kernel into TileContext tc (single SPMD program, all cores)."""
    import concourse.bass as bass
    from concourse import mybir

    f32 = mybir.dt.float32
    f16 = mybir.dt.float16
    AF = mybir.ActivationFunctionType
    OP = mybir.AluOpType
    evs = _schedule()

    initT = nc.dram_tensor("initT", [KIN, NB], f16, kind="ExternalInput").ap()
    epsT = nc.dram_tensor("epsT", [DIM, NB], f16, kind="ExternalInput").ap()
    onesZ_d = nc.dram_tensor("onesZ", [DIM, 1], f16, kind="ExternalInput").ap()
    W1v_d = nc.dram_tensor("W1v", [KIN, NEV * NCH, P], f16, kind="ExternalInput").ap()
    W2T_d = nc.dram_tensor("W2T", [DIM, HID], f16, kind="ExternalInput").ap()
    W2fa_d = nc.dram_tensor("W2fa", [P, NEV, NCH, CW], f16, kind="ExternalInput").ap()
    divW_d = nc.dram_tensor("divW", [P, NEV, 1], f16, kind="ExternalInput").ap()
    W1z_d = nc.dram_tensor("W1z", [DIM, HID], f16, kind="ExternalInput").ap()
    b2c_d = nc.dram_tensor("b2c", [DIM, 1], f32, kind="ExternalInput").ap()
    out_d = nc.dram_tensor("out", [1, NB], f32, kind="ExternalOutput").ap()

    const = ctx.enter_context(tc.tile_pool(name="const", bufs=1))
    state = ctx.enter_context(tc.tile_pool(name="state", bufs=1))
    work = ctx.enter_context(tc.tile_pool(name="work", bufs=4))
    pa_pool = ctx.enter_context(tc.tile_pool(name="pa", bufs=1, space="PSUM"))
    fd_pool = ctx.enter_context(tc.tile_pool(name="fd", bufs=3, space="PSUM"))

    # ---- persistent SBUF ----
    inT = state.tile([KIN, NB], f16)
    zS = state.tile([SD_P, NB], f16)     # rows 0-15 z, row 16 logp
    zSacc = state.tile([SD_P, NB], f16)  # in-step RK4 accumulator
    u = state.tile([P, NCH, NB], f16)
    outF = state.tile([1, NB], f32)
    W1v = const.tile([KIN, NEV * NCH, P], f16)
    W2T = const.tile([DIM, HID], f16)
    W2fa = const.tile([P, NEV, NCH, CW], f16)
    divW = const.tile([P, NEV, 1], f16)
    W1z = const.tile([DIM, HID], f16)
    ones16 = const.tile([P, 1], f16)
    onesZ = const.tile([DIM, 1], f16)
    b2c = const.tile([DIM, 1], f32)
    ept = const.tile([DIM, NB], f16)

    nc.gpsimd.dma_start(ept[:], epsT)
    nc.gpsimd.dma_start(W1z[:], W1z_d)
    nc.gpsimd.dma_start(W2T[:], W2T_d)
    nc.gpsimd.dma_start(W1v[:], W1v_d)
    nc.gpsimd.dma_start(inT[:, :], initT)
    nc.gpsimd.dma_start(onesZ[:], onesZ_d)
    nc.vector.memset(zS[0:SD_P, :], 0.0)
    nc.gpsimd.dma_start(zS[0:DIM, :], initT[0:DIM, :])
    nc.gpsimd.dma_start(W2fa[:], W2fa_d)
    nc.gpsimd.dma_start(divW[:], divW_d)
    nc.gpsimd.dma_start(b2c[:], b2c_d)
    nc.vector.memset(ones16[:], 1.0)

    # ---- precompute u = (eps@W1z) * (eps@W2^T), transposed layout ----
    def emit_pre(qt, c):
        js = slice(qt * (NB // 4), (qt + 1) * (NB // 4))
        tv = pa_pool.tile([P, 4, 512], f32, tag="pa")
        for n in range(2):
            cs = slice((qt * 2 + n) * 512, (qt * 2 + n + 1) * 512)
            nc.tensor.matmul(
                tv[:, n, :], W1z[:, c * P : (c + 1) * P], ept[:, cs],
                start=True, stop=True,
            )
            nc.tensor.matmul(
                tv[:, 2 + n, :], W2T[:, c * P : (c + 1) * P], ept[:, cs],
                start=True, stop=True,
            )
        usl = u[:, c, js].rearrange("p (a b) -> p a b", a=2)
        nc.scalar.activation(usl, tv[:, 0:2, :], AF.Copy)
        nc.vector.tensor_tensor(usl, usl, tv[:, 2:4, :], op=OP.mult)

    def emit_colsum(j):
        # U = colsum(u) -> zS row 32 = U - 0.5*DIM*log(2pi)
        js = slice(j * 512, (j + 1) * 512)
        pU = fd_pool.tile([1, 512], f32, tag="fd")
        for c in range(NCH):
            nc.tensor.matmul(
                pU[:, :], ones16[:], u[:, c, js], start=(c == 0), stop=(c == NCH - 1)
            )
        nc.scalar.activation(
            zS[DV : DV + 1, js], pU[:, :], AF.Copy, bias=-0.5 * DIM * LOG2PI
        )

    for qt in range(4):
        for c in range(NCH):
            emit_pre(qt, c)
    for j in range(NJ):
        emit_colsum(j)

    # ---- main loop: flat software pipeline over 128 unit-evals ----
    units = []
    for s in range(NSTEPS):
        for hb in range(2):
            for stage in range(NSTAGE):
                for uu in range(4):
                    units.append((s, hb, stage, uu))

    def emit_front(k, s, hb, stage, uu):
        i = s * NSTAGE + stage
        j = hb * 4 + uu
        js = slice(j * 512, (j + 1) * 512)
        if uu % 2 == 0:
            emit_front.cur = fd_pool.tile([97, 512], f32, tag="fd")
        cur = emit_front.cur
        base = 64 * (uu % 2)
        pa = pa_pool.tile([P, 4, 512], f32, tag="pa")
        for c in range(NCH):
            nc.tensor.matmul(pa[:, c, :], W1v[:, i * NCH + c, :], inT[:, js],
                             start=True, stop=True)
        h = work.tile([P, NCH, 512], f16, tag="h", bufs=3)
        nc.scalar.activation(h[:, :, :], pa[:, :, :], AF.Tanh)
        q = work.tile([P, NCH, 512], f16, tag="q", bufs=3)
        if stage != NSTAGE - 1:
            nc.gpsimd.tensor_tensor(q[:, 3, :], h[:, 3, :], h[:, 3, :], op=OP.mult)
            nc.vector.tensor_tensor(q[:, 0:3, :], h[:, 0:3, :], h[:, 0:3, :],
                                    op=OP.mult)
        else:
            nc.vector.tensor_tensor(q[:, :, :], h[:, :, :], h[:, :, :], op=OP.mult)
        nc.vector.tensor_tensor(q[:, :, :], q[:, :, :], u[:, :, js], op=OP.mult)
        return (k, evs[i], i, s, stage, js, cur, base, h, q)

    def emit_back(pk):
        k, ev, i, s, stage, js, cur, base, h, q = pk
        for c in range(NCH):
            nc.tensor.matmul(
                cur[base : base + CW, :], W2fa[:, i, c, :], h[:, c, :],
                start=(c == 0), stop=False, skip_group_check=True,
            )
        for c in range(NCH):
            nc.tensor.matmul(
                cur[base + DV : base + DV + 1, :], divW[:, i, :], q[:, c, :],
                start=False, stop=(c == NCH - 1), skip_group_check=True,
                tile_position=(0, base + DV),
            )
        # CUR psum -> sbuf copy; rotate some copies to DVE for engine balance
        curS = work.tile([CW, 512], f16, tag="cs", bufs=4)
        if k % 8 == 5:
            nc.vector.tensor_scalar_add(curS[:, :], cur[base : base + CW, :], 0.0)
        else:
            nc.scalar.activation(curS[:, :], cur[base : base + CW, :], AF.Copy)
        if ev["alpha"] is not None:  # stages 0-2
            # next-stage input: z' = zS + (alpha/w) * (w*f)
            nc.vector.scalar_tensor_tensor(
                inT[0:DIM, js], curS[0:DIM, :], ev["alpha"] / ev["w"],
                zS[0:DIM, js], op0=OP.mult, op1=OP.add,
            )
            src = zS if stage == 0 else zSacc
            nc.gpsimd.tensor_tensor(
                zSacc[:, js], curS[0:SD_P, :], src[:, js], op=OP.add
            )
        else:  # stage 3: fold step-end into the adds (all partition-aligned)
            if s != NSTEPS - 1:
                nc.gpsimd.tensor_tensor(
                    inT[0:DIM, js], curS[0:DIM, :], zSacc[0:DIM, js], op=OP.add
                )
            nc.gpsimd.tensor_tensor(
                zS[:, js], curS[0:SD_P, :], zSacc[:, js], op=OP.add
            )

    from collections import deque
    pend = deque()
    for k, (s, hb, stage, uu) in enumerate(units):
        pend.append(emit_front(k, s, hb, stage, uu))
        if len(pend) > 2:
            emit_back(pend.popleft())
    while pend:
        emit_back(pend.popleft())

    # ---- finalize: out = -0.5*sum(z1^2) - 0.5*D*log(2pi) + delta_logp ----
    z1 = ept
    nc.vector.tensor_scalar(z1[:, :], zS[0:DIM, :], b2c[:], None, op0=OP.add)
    zsq = ept
    nc.vector.tensor_tensor(zsq[:, :], z1[:, :], z1[:, :], op=OP.mult)
    outr = outF[0:1, :]
    for j in range(NJ):
        js = slice(j * 512, (j + 1) * 512)
        pZ = fd_pool.tile([1, 512], f32, tag="fd")
        nc.tensor.matmul(pZ[:, :], onesZ[:], zsq[:, js], start=True, stop=True)
        nc.vector.scalar_tensor_tensor(
            outr[:, js], pZ[:, :], -0.5, zS[DV : DV + 1, js],
            op0=OP.mult, op1=OP.add,
        )
    nc.gpsimd.dma_start(out_d, outr)


_COMPILED = {}


def _get_compiled():
    if "nc" in _COMPILED:
        return _COMPILED["nc"]
    from contextlib import ExitStack
    import concourse.tile as tile
    from concourse import bacc

    nc = bacc.Bacc("TRN2", target_bir_lowering=False, debug=False,
                   num_devices=NCORES)
    with tile.TileContext(nc) as tc, ExitStack() as ctx:
        build(nc, tc, ctx)
    nc.compile()
    _COMPILED["nc"] = nc
    return nc


def kernel(x, context, eps, W1, b1, W2, b2, steps):
    from concourse.bass_utils import run_bass_kernel_spmd

    assert int(steps) == 5, "kernel hardcodes the steps=5 schedule"
    in_maps = prep_host_inputs(x, context, eps, W1, b1, W2, b2)
    nc = _get_compiled()
    res = run_bass_kernel_spmd(nc, in_maps, list(range(NCORES)))
    out = np.concatenate(
        [res.results[i]["out"].reshape(NB, 1) for i in range(NCORES)], axis=0
    )
    return out.astype(np.float32)


if __name__ == "__main__":
    rng = np.random.default_rng(0)
    ins = dict(
        x=rng.standard_normal((B, DIM), dtype=np.float32),
        context=rng.standard_normal((B, COND), dtype=np.float32),
        eps=rng.standard_normal((B, DIM), dtype=np.float32),
        W1=(rng.standard_normal((KIN, HID)) / np.sqrt(KIN)).astype(np.float32),
        b1=np.zeros(HID, np.float32),
        W2=(rng.standard_normal((HID, DIM)) / np.sqrt(HID)).astype(np.float32),
        b2=np.zeros(DIM, np.float32),
        steps=5,
    )
    print(kernel(**ins)[:4])


# revision 16
# speedup vs baseline: 1.1545x; 1.1545x over previous
"""Trainium2 Bass kernel for CNF log-prob (nn_CNF_86019605004441).

Reference computation (per batch row b of B=32768):
  Integrate (z, logp) from t=1 to t=0 with 4 fixed RK4 steps (steps=5 ->
  4 intervals). Each RK4 stage evaluates
     f(t, z)   = tanh([z, ctx, t] @ W1 + b1) @ W2 + b2
     div(t, z) = eps^T J eps  (Hutchinson, exact via jvp)
  With h = tanh(a):  div = sum_j (1 - h_j^2) * t1_j * v_j
     where t1 = eps @ W1[:16]  and  v = eps @ W2^T  are eval-independent.
  Using u = t1*v and U = sum_j u_j:  div = U - S,  S = sum_j h_j^2 u_j.
  logp(x) = -0.5*sum(z1^2) - 0.5*16*log(2pi) + delta_logp.

Sharding: pure data parallel, batch 32768 -> 8 cores x 4096 rows.

On-core layout (features on partitions, batch on the free axis, fp16 state):
  inT [81, 4096] f16: rows 0-15 z (current eval input), 16-79 ctx, 80 ones.
  mm1 stationary per (eval i, chunk c): W1v[:, i*4+c, :] [81,128] f16; row 80
      carries beta = t_i*W1_time + b1 + delta_i*(W1z.T@b2) (time feature, b1,
      deferred-b2 folded in -> ACT does a pure tanh).
  zS/zSacc [17, 4096] f16: rows 0-15 z state, row 16 logp (init
      U - 0.5*16*log(2pi); the -dt*U divergence constant telescopes to +U).

Main loop: one flat software pipeline over 128 unit-evals E_k =
  (step s, half hb, stage, unit uu) of 512 batch cols each. Per E_k:
    PE : 4x mm1 into pa psum (2 tiles of [128,2,512]);  4x f-mm2 with a
         STACKED stationary [w*W2 | 0 0 | alpha*W2] ([128,34] f16) + 4x
         div-mm2 ([w, alpha] cols) -> one CUR psum region of 34 rows
         (0-15 w*f, 16 w*S, 17 a*S, 18-33 a*f), 2 units/bank (base 0/64).
    ACT: 2x tanh [128,2,512] psum->sbuf f16; 1x Copy CUR [34,512] -> curS
         (some rotated to DVE for balance).
    DVE: hh = h*h, q = hh*u in place (fp16 2x-mode); inT z-rows =
         curS[18:34] + zS (the one partition-offset read, DVE-proven).
    Pool(GPSIMD, sbuf-only, partition-aligned only):
         zSacc[0:17] = curS[0:17] + (zS | zSacc)     [w-scaled RK4 accum]
         stage3: zS = curS[0:17] + zSacc, inT = curS[0:16] + zSacc.
  Emission interleaves E_k's mm1/tanh/hh/q with E_{k-1}'s f/div/copy/adds so
  every engine queue stays one unit ahead of its dependencies.
Finalize: zsq = (z1 - b2)^2 ; colsum via ones-matmul ; out = -0.5*colsum
  + zS[16].
"""

import sys
import numpy as np

for _p in ("/opt/trn_rl_repo",):
    if _p not in sys.path:
        sys.path.insert(0, _p)

DIM, COND, HID = 16, 64, 512
B, NCORES = 32768, 8
NB = B // NCORES          # 4096 batch rows per core
P = 128                   # partitions
NCH = HID // P            # 4 hidden chunks
NJ = NB // 512            # 8 batch column groups
KIN = DIM + COND + 1      # 81 stationary rows (z, ctx, ones)
ONE_R = KIN - 1           # 80 = ones/beta row
SD_P = 2 * DIM + 1        # 33 = state rows (z 0-15, zero pad, logp 32)
DV = 2 * DIM              # 32 = logp row
CW = 2 * DIM + 1          # 33 = CUR region rows (w*f 0-15, pad, w*S 32)
NSTEPS, NSTAGE = 4, 4
NEV = NSTEPS * NSTAGE     # 16 rhs evaluations
LOG2PI = float(np.log(2.0 * np.pi))


def _schedule():
    """Per-eval (t, alpha_next, w, delta) for classic RK4, t:1->0, dt=-0.25."""
    ts = np.linspace(1.0, 0.0, NSTEPS + 1)
    evs = []
    for s in range(NSTEPS):
        t0 = float(ts[s])
        dt = float(ts[s + 1] - ts[s])
        dbase = s * dt
        evs.append(dict(t=t0, alpha=dt / 2, w=dt / 6, delta=dbase))
        evs.append(dict(t=t0 + dt / 2, alpha=dt / 2, w=dt / 3, delta=dbase + dt / 2))
        evs.append(dict(t=t0 + dt / 2, alpha=dt, w=dt / 3, delta=dbase + dt / 2))
        evs.append(dict(t=t0 + dt, alpha=None, w=dt / 6, delta=dbase + dt))
    return evs


def prep_host_inputs(x, context, eps, W1, b1, W2, b2):
    """Host-side layout prep (transposes + per-eval stationary weight packing).

    Returns the in_map dict for one core given that core's batch slice."""
    evs = _schedule()
    W1 = np.asarray(W1, np.float32)
    b1 = np.asarray(b1, np.float32)
    W2 = np.asarray(W2, np.float32)
    b2 = np.asarray(b2, np.float32)

    gz = W1[:DIM].T @ b2  # [512], the z-column correction for deferred b2
    W1v = np.zeros((KIN, NEV * NCH, P), np.float32)
    for i, ev in enumerate(evs):
        for c in range(NCH):
            sl = slice(c * P, (c + 1) * P)
            v = i * NCH + c
            W1v[0:DIM, v, :] = W1[0:DIM, sl]
            W1v[DIM:ONE_R, v, :] = W1[DIM : DIM + COND, sl]
            W1v[ONE_R, v, :] = (
                ev["t"] * W1[DIM + COND, sl] + b1[sl] + ev["delta"] * gz[sl]
            )

    # f-mm2 stationary per eval, w_i prescaled (cols 0-15 w_i*W2, cols
    # 16-32 zero); div-mm2 one col [w_i] -> psum row 32 (32-aligned out).
    W2fa = np.zeros((P, NEV, NCH, CW), np.float16)
    divW = np.zeros((P, NEV, 1), np.float16)
    w2c = W2.reshape(NCH, P, DIM).transpose(1, 0, 2)  # [P, NCH, DIM]
    for i, ev in enumerate(evs):
        W2fa[:, i, :, 0:DIM] = (w2c * ev["w"]).astype(np.float16)
        divW[:, i, 0] = np.float16(ev["w"])

    W2T = np.ascontiguousarray(W2.T)  # [16, 512] for the v = eps@W2^T matmul
    b2c = (NSTEPS * (-0.25)) * b2.reshape(DIM, 1).astype(np.float32)

    def core_map(xs, cs, es):
        initT = np.zeros((KIN, NB), np.float32)
        initT[0:DIM] = xs.T
        initT[DIM:ONE_R] = cs.T
        initT[ONE_R] = 1.0
        return {
            "initT": initT.astype(np.float16),             # [81, NB]
            "epsT": np.ascontiguousarray(es.T).astype(np.float16),
            "onesZ": np.ones((DIM, 1), np.float16),
            "W1v": W1v.astype(np.float16),                 # [81, 64, 128]
            "W2T": W2T.astype(np.float16),                 # [16, 512]
            "W1z": W1[0:DIM].astype(np.float16),           # [16, 512]
            "W2fa": W2fa,                                  # [128, 16, 4, 34]
            "divW": divW,                                  # [128, 16, 1]
            "b2c": b2c,                                    # [16, 1]
        }

    return [
        core_map(
            np.asarray(x, np.float32)[i * NB : (i + 1) * NB],
            np.asarray(context, np.float32)[i * NB : (i + 1) * NB],
            np.asarray(eps, np.float32)[i * NB : (i + 1) * NB],
        )
        for i in range(NCORES)
    ]


def build(nc, tc, ctx):
    """Emit the kernel into TileContext tc (single SPMD program, all cores)."""
    import concourse.bass as bass
    from concourse import mybir

    f32 = mybir.dt.float32
    f16 = mybir.dt.float16
    AF = mybir.ActivationFunctionType
    OP = mybir.AluOpType
    evs = _schedule()

    initT = nc.dram_tensor("initT", [KIN, NB], f16, kind="ExternalInput").ap()
    epsT = nc.dram_tensor("epsT", [DIM, NB], f16, kind="ExternalInput").ap()
    onesZ_d = nc.dram_tensor("onesZ", [DIM, 1], f16, kind="ExternalInput").ap()
    W1v_d = nc.dram_tensor("W1v", [KIN, NEV * NCH, P], f16, kind="ExternalInput").ap()
    W2T_d = nc.dram_tensor("W2T", [DIM, HID], f16, kind="ExternalInput").ap()
    W2fa_d = nc.dram_tensor("W2fa", [P, NEV, NCH, CW], f16, kind="ExternalInput").ap()
    divW_d = nc.dram_tensor("divW", [P, NEV, 1], f16, kind="ExternalInput").ap()
    W1z_d = nc.dram_tensor("W1z", [DIM, HID], f16, kind="ExternalInput").ap()
    b2c_d = nc.dram_tensor("b2c", [DIM, 1], f32, kind="ExternalInput").ap()
    out_d = nc.dram_tensor("out", [1, NB], f32, kind="ExternalOutput").ap()

    const = ctx.enter_context(tc.tile_pool(name="const", bufs=1))
    state = ctx.enter_context(tc.tile_pool(name="state", bufs=1))
    work = ctx.enter_context(tc.tile_pool(name="work", bufs=4))
    pa_pool = ctx.enter_context(tc.tile_pool(name="pa", bufs=2, space="PSUM"))
    fd_pool = ctx.enter_context(tc.tile_pool(name="fd", bufs=4, space="PSUM"))

    # ---- persistent SBUF ----
    inT = state.tile([KIN, NB], f16)
    zS = state.tile([SD_P, NB], f16)     # rows 0-15 z, row 16 logp
    zSacc = state.tile([SD_P, NB], f16)  # in-step RK4 accumulator
    u = state.tile([P, NCH, NB], f16)
    outF = state.tile([1, NB], f32)
    W1v = const.tile([KIN, NEV * NCH, P], f16)
    W2T = const.tile([DIM, HID], f16)
    W2fa = const.tile([P, NEV, NCH, CW], f16)
    divW = const.tile([P, NEV, 1], f16)
    W1z = const.tile([DIM, HID], f16)
    ones16 = const.tile([P, 1], f16)
    onesZ = const.tile([DIM, 1], f16)
    b2c = const.tile([DIM, 1], f32)
    ept = const.tile([DIM, NB], f16)

    nc.gpsimd.dma_start(ept[:], epsT)
    nc.gpsimd.dma_start(W1z[:], W1z_d)
    nc.gpsimd.dma_start(W2T[:], W2T_d)
    nc.gpsimd.dma_start(W1v[:], W1v_d)
    nc.gpsimd.dma_start(inT[:, :], initT)
    nc.gpsimd.dma_start(onesZ[:], onesZ_d)
    nc.vector.memset(zS[0:SD_P, :], 0.0)
    nc.gpsimd.dma_start(zS[0:DIM, :], initT[0:DIM, :])
    nc.gpsimd.dma_start(W2fa[:], W2fa_d)
    nc.gpsimd.dma_start(divW[:], divW_d)
    nc.gpsimd.dma_start(b2c[:], b2c_d)
    nc.vector.memset(ones16[:], 1.0)

    # ---- precompute u = (eps@W1z) * (eps@W2^T), transposed layout ----
    def emit_pre(qt, c, n):
        # one 512-col half: t1 into bank 0, v into bank 1 of one pa buf
        jn = qt * 2 + n
        cs = slice(jn * 512, (jn + 1) * 512)
        t = pa_pool.tile([P, 2, 512], f32, tag="pa")
        nc.tensor.matmul(
            t[:, 0, :], W1z[:, c * P : (c + 1) * P], ept[:, cs],
            start=True, stop=True,
        )
        nc.tensor.matmul(
            t[:, 1, :], W2T[:, c * P : (c + 1) * P], ept[:, cs],
            start=True, stop=True,
        )
        usl = u[:, c, cs]
        nc.scalar.activation(usl, t[:, 0, :], AF.Copy)
        nc.vector.tensor_tensor(usl, usl, t[:, 1, :], op=OP.mult)

    def emit_colsum(j):
        # U = colsum(u) -> zS row 32 = U - 0.5*DIM*log(2pi)
        js = slice(j * 512, (j + 1) * 512)
        pU = fd_pool.tile([1, 512], f32, tag="fd")
        for c in range(NCH):
            nc.tensor.matmul(
                pU[:, :], ones16[:], u[:, c, js], start=(c == 0), stop=(c == NCH - 1)
            )
        nc.scalar.activation(
            zS[DV : DV + 1, js], pU[:, :], AF.Copy, bias=-0.5 * DIM * LOG2PI
        )

    for qt in range(2):
        for c in range(NCH):
            emit_pre(qt, c, 0)
            emit_pre(qt, c, 1)
    for j in range(4):
        emit_colsum(j)
    deferred = [(qt, c, n) for qt in (2, 3) for c in range(NCH) for n in range(2)]

    # ---- main loop: flat software pipeline over 128 unit-evals ----
    units = []
    for s in range(NSTEPS):
        for hb in range(2):
            for stage in range(NSTAGE):
                for uu in range(4):
                    units.append((s, hb, stage, uu))

    def emit_front(k, s, hb, stage, uu):
        i = s * NSTAGE + stage
        j = hb * 4 + uu
        js = slice(j * 512, (j + 1) * 512)
        if uu % 2 == 0:
            emit_front.cur = fd_pool.tile([97, 512], f32, tag="fd")
        cur = emit_front.cur
        base = 64 * (uu % 2)
        paA = pa_pool.tile([P, 2, 512], f32, tag="pa")
        nc.tensor.matmul(paA[:, 0, :], W1v[:, i * NCH + 0, :], inT[:, js],
                         start=True, stop=True)
        nc.tensor.matmul(paA[:, 1, :], W1v[:, i * NCH + 1, :], inT[:, js],
                         start=True, stop=True)
        paB = pa_pool.tile([P, 2, 512], f32, tag="pa")
        nc.tensor.matmul(paB[:, 0, :], W1v[:, i * NCH + 2, :], inT[:, js],
                         start=True, stop=True)
        nc.tensor.matmul(paB[:, 1, :], W1v[:, i * NCH + 3, :], inT[:, js],
                         start=True, stop=True)
        h = work.tile([P, NCH, 512], f16, tag="h", bufs=3)
        nc.scalar.activation(h[:, 0:2, :], paA[:, :, :], AF.Tanh)
        nc.scalar.activation(h[:, 2:4, :], paB[:, :, :], AF.Tanh)
        q = work.tile([P, NCH, 512], f16, tag="q", bufs=3)
        if stage != NSTAGE - 1:
            nc.gpsimd.tensor_tensor(q[:, 3, :], h[:, 3, :], h[:, 3, :], op=OP.mult)
            nc.vector.tensor_tensor(q[:, 0:3, :], h[:, 0:3, :], h[:, 0:3, :],
                                    op=OP.mult)
        else:
            nc.vector.tensor_tensor(q[:, :, :], h[:, :, :], h[:, :, :], op=OP.mult)
        nc.vector.tensor_tensor(q[:, :, :], q[:, :, :], u[:, :, js], op=OP.mult)
        return (k, evs[i], i, s, stage, js, cur, base, h, q)

    def emit_back(pk):
        k, ev, i, s, stage, js, cur, base, h, q = pk
        for c in range(NCH):
            nc.tensor.matmul(
                cur[base : base + CW, :], W2fa[:, i, c, :], h[:, c, :],
                start=(c == 0), stop=False, skip_group_check=True,
            )
        for c in range(NCH):
            nc.tensor.matmul(
                cur[base + DV : base + DV + 1, :], divW[:, i, :], q[:, c, :],
                start=False, stop=(c == NCH - 1), skip_group_check=True,
                tile_position=(0, base + DV),
            )
        # CUR psum -> sbuf copy; rotate some copies to DVE for engine balance
        curS = work.tile([CW, 512], f16, tag="cs", bufs=4)
        if k % 8 == 5:
            nc.vector.tensor_scalar_add(curS[:, :], cur[base : base + CW, :], 0.0)
        else:
            nc.scalar.activation(curS[:, :], cur[base : base + CW, :], AF.Copy)
        if ev["alpha"] is not None:  # stages 0-2
            # next-stage input: z' = zS + (alpha/w) * (w*f)
            nc.vector.scalar_tensor_tensor(
                inT[0:DIM, js], curS[0:DIM, :], ev["alpha"] / ev["w"],
                zS[0:DIM, js], op0=OP.mult, op1=OP.add,
            )
            src = zS if stage == 0 else zSacc
            nc.gpsimd.tensor_tensor(
                zSacc[:, js], curS[0:SD_P, :], src[:, js], op=OP.add
            )
        else:  # stage 3: fold step-end into the adds (all partition-aligned)
            if s != NSTEPS - 1:
                nc.gpsimd.tensor_tensor(
                    inT[0:DIM, js], curS[0:DIM, :], zSacc[0:DIM, js], op=OP.add
                )
            nc.gpsimd.tensor_tensor(
                zS[:, js], curS[0:SD_P, :], zSacc[:, js], op=OP.add
            )

    from collections import deque
    pend = deque()
    for k, (s, hb, stage, uu) in enumerate(units):
        pend.append(emit_front(k, s, hb, stage, uu))
        if k < len(deferred):
            emit_pre(*deferred[k])
            if k == len(deferred) - 1:
                for j in range(4, NJ):
                    emit_colsum(j)
        if len(pend) > 2:
            emit_back(pend.popleft())
    while pend:
        emit_back(pend.popleft())

    # ---- finalize: out = -0.5*sum(z1^2) - 0.5*D*log(2pi) + delta_logp ----
    z1 = ept
    nc.vector.tensor_scalar(z1[:, :], zS[0:DIM, :], b2c[:], None, op0=OP.add)
    zsq = ept
    nc.vector.tensor_tensor(zsq[:, :], z1[:, :], z1[:, :], op=OP.mult)
    outr = outF[0:1, :]
    for j in range(NJ):
        js = slice(j * 512, (j + 1) * 512)
        pZ = fd_pool.tile([1, 512], f32, tag="fd")
        nc.tensor.matmul(pZ[:, :], onesZ[:], zsq[:, js], start=True, stop=True)
        nc.vector.scalar_tensor_tensor(
            outr[:, js], pZ[:, :], -0.5, zS[DV : DV + 1, js],
            op0=OP.mult, op1=OP.add,
        )
    nc.gpsimd.dma_start(out_d, outr)


_COMPILED = {}


def _get_compiled():
    if "nc" in _COMPILED:
        return _COMPILED["nc"]
    from contextlib import ExitStack
    import concourse.tile as tile
    from concourse import bacc

    nc = bacc.Bacc("TRN2", target_bir_lowering=False, debug=False,
                   num_devices=NCORES)
    with tile.TileContext(nc) as tc, ExitStack() as ctx:
        build(nc, tc, ctx)
    nc.compile()
    _COMPILED["nc"] = nc
    return nc


def kernel(x, context, eps, W1, b1, W2, b2, steps):
    from concourse.bass_utils import run_bass_kernel_spmd

    assert int(steps) == 5, "kernel hardcodes the steps=5 schedule"
    in_maps = prep_host_inputs(x, context, eps, W1, b1, W2, b2)
    nc = _get_compiled()
    res = run_bass_kernel_spmd(nc, in_maps, list(range(NCORES)))
    out = np.concatenate(
        [res.results[i]["out"].reshape(NB, 1) for i in range(NCORES)], axis=0
    )
    return out.astype(np.float32)


if __name__ == "__main__":
    rng = np.random.default_rng(0)
    ins = dict(
        x=rng.standard_normal((B, DIM), dtype=np.float32),
        context=rng.standard_normal((B, COND), dtype=np.float32),
        eps=rng.standard_normal((B, DIM), dtype=np.float32),
        W1=(rng.standard_normal((KIN, HID)) / np.sqrt(KIN)).astype(np.float32),
        b1=np.zeros(HID, np.float32),
        W2=(rng.standard_normal((HID, DIM)) / np.sqrt(HID)).astype(np.float32),
        b2=np.zeros(DIM, np.float32),
        steps=5,
    )
    print(kernel(**ins)[:4])
